# revision 16
# baseline (speedup 1.0000x reference)
"""Trainium2 Bass kernel for the APheSCL supervised-contrastive loss.

Data-parallel over anchor rows of the [N,N] logits matrix (N=V*B=4096),
256 batch rows (512 anchors) per core. v2 redesign vs the first working
kernel (148us):

 - fp16 matmuls (PE 1 cycle/row vs fp32's 4) for the N^2 gram pass.
 - softmax denominator: per 128-anchor row-block, 4 matmuls fill a
   [128,2048] PSUM span (4 banks) and ONE activation instruction does
   exp((adc-1)/T) with accum_out -> 8 big ACT ops total instead of 64
   small ones (ACT SBUF-access init is ~185ns per instruction).
 - the sim-weighted logit sum S2 = sum_j sim[a,j]*adc[a,j] is computed
   as g_a . P_a with P = Sim @ [H | 1], H_j = g_j^v0 + g_j^v1 - a tiny
   PE matmul - instead of 32 DVE scalar_tensor_tensor passes. The ones
   column gives S3 = sim row sums for free.
 - HOST-side prep (layout only, O(N*D)): rows are SORTED by the packed
   phenotype key (label + cats). sim[a,j] is nonzero only where keys
   match (plus ~e^-32 tails), so each sorted 128-anchor block's support
   is one contiguous j-window; the kernel computes sim on [128, W]
   windows (W ~ 256-512) instead of [128, 2048] - ~4-8x less DVE work.
   Window tables/H slices are gathered per core on host so the SPMD
   program stays static.
 - key equality folded into the L1 distance as a +64*(key!=mykey)
   pseudo-feature (exp(-32) ~ 1e-14 kills mismatches); |dx| built from
   validated DVE ops: d=TS(sub) then |d|=STT(d,-1,d,mult,max).
 - sim^T for the P-matmul via one 3D XBAR DMA-transpose per k-block
   (out[j,jb,a] = in[a, jb*128+j], verified on HW), zero engine time.
 - diagonal exp terms (exact, from the same fp16 values the PE saw) are
   subtracted from the denominator; same-view diag of S2 corrected to
   the reference's clip(adc)=1.

Host does normalization + fp16 cast + transposes + key packing/sorting
(O(N*D) layout prep); every O(N^2) term runs on device.
"""

import os

import numpy as np

TEMP = 0.07
INV_T = 1.0 / TEMP
EPS = 1e-8
B = 2048
V = 2
D = 128
N = V * B
NCORES = 8
RB = B // NCORES          # 256 batch rows per core
NK = RB // 128            # 2 anchor k-blocks of 128 per core
NRB = V * NK              # 4 anchor row-blocks per core (view-major)
CT = 512                  # matmul moving tile
DEN_CHUNK = 2048          # PSUM span per den exp instruction
NCH = N // DEN_CHUNK      # 2 chunks per row-block


def _patch_tile_drain():
    """This container's walrus rejects >1 sync-wait on one TPB_CTRL
    (Drain). Split the TileContext tail-drain's waits across single-wait
    SP nops (still before the all-engine barrier: semantics unchanged)."""
    from concourse import tile, mybir
    from concourse.vector_clock import ScopedClock

    if getattr(tile.TileContext, "_drain_split_patched", False):
        return

    def _drain_and_barrier(self, tick_clock, wait_clock):
        nc = self.nc
        drain_inst = nc.sync.drain()
        wait_clock.add_sem_waits(
            drain_inst.ins, ScopedClock({None: tick_clock.global_clock})
        )
        si = drain_inst.ins.sync_info
        if si is not None and si.on_wait and len(si.on_wait) > 1:
            waits = list(si.on_wait)
            si.on_wait = waits[:1]
            for w in waits[1:]:
                nop = nc.sync.nop(nofuse=True, hint="drain_split_wait")
                nsi = nop.ins.sync_info
                if nsi is None:
                    nop.ins.sync_info = mybir.SyncInfo(on_wait=[w], on_update=[])
                else:
                    nsi.on_wait = [w]
        nc.all_engine_barrier()
        assert self.sems is not None
        popped = nc._tile_sem_poison_stack.pop()
        assert popped is self._sem_poison
        nc.clear_and_free_semaphores(list(self.sems.allocated().values()))
        nc.all_engine_barrier()

    tile.TileContext._drain_and_barrier = _drain_and_barrier
    tile.TileContext._drain_split_patched = True


_MAXW = 1


def _split_waits(nc, maxw=_MAXW):
    """This walrus build rejects instructions carrying more than ~1 sync
    wait (and 0 on the DMA-transpose struct). Move excess waits onto
    same-engine nops inserted immediately before the offending
    instruction (same program point -> semantics unchanged). Engine nops
    don't flush the datapath pipeline, unlike the drains used before."""
    from concourse import mybir

    eng_map = {
        mybir.EngineType.PE: nc.tensor,
        mybir.EngineType.DVE: nc.vector,
        mybir.EngineType.Activation: nc.scalar,
        mybir.EngineType.Pool: nc.gpsimd,
        mybir.EngineType.SP: nc.sync,
    }
    for f in nc.m.functions:
        for bb in f.blocks:
            insts = bb.instructions
            i = 0
            while i < len(insts):
                ins = insts[i]
                si = ins.sync_info
                eng = getattr(ins, "engine", None)
                mw = 0 if type(ins).__name__ == "InstDmaTransposeAnt" else maxw
                if (si is not None and si.on_wait and len(si.on_wait) > mw
                        and eng in eng_map):
                    waits = list(si.on_wait)
                    si.on_wait = waits[-mw:] if mw else []
                    extra = waits[:-mw] if mw else waits
                    pre = []
                    step = max(maxw, 1)
                    for k in range(0, len(extra), step):
                        nop = eng_map[eng].nop(nofuse=True)
                        nop_ins = nop.ins
                        for fb in f.blocks:
                            if fb.instructions and fb.instructions[-1] is nop_ins:
                                fb.instructions.pop()
                                break
                        nop_ins.sync_info = mybir.SyncInfo(
                            on_wait=list(extra[k : k + step]), on_update=[])
                        pre.append(nop_ins)
                    for off, p in enumerate(pre):
                        insts.insert(i + off, p)
                    i += len(pre)
                i += 1


def _build(wfix):
    nc = _build_inner(wfix)
    _split_waits(nc)
    return nc


def _build_inner(wfix):
    from concourse import bass, tile, mybir

    _patch_tile_drain()
    f32 = mybir.dt.float32
    f16 = mybir.dt.float16
    Alu = mybir.AluOpType
    Act = mybir.ActivationFunctionType

    WB = wfix // 128

    nc = bass.Bass("TRN2", target_bir_lowering=False, debug=False,
                   num_devices=NCORES)

    gt16 = nc.declare_dram_parameter("gt16", [128, N], f16, isOutput=False)
    mygt = nc.declare_dram_parameter("mygt", [128, NRB, 128], f16, isOutput=False)
    myg = nc.declare_dram_parameter("myg", [128, NRB, 128], f16, isOutput=False)
    wh = nc.declare_dram_parameter("wh", [128, NK, WB, 129], f16, isOutput=False)
    # all 10 window-table rows (per k: 4 cont + keyid*32) in one fp16 param
    wtab = nc.declare_dram_parameter("wtab", [1, NK * 5 * wfix], f16, isOutput=False)
    wmy = nc.declare_dram_parameter("wmy", [128, NK, 5], f32, isOutput=False)
    out_ext = nc.declare_dram_parameter("out", [128, 3, NRB], f32, isOutput=True)

    with tile.TileContext(nc) as tc:
        with (
            tc.tile_pool(name="persist", bufs=1) as pp,
            tc.tile_pool(name="work", bufs=2) as wp,
            tc.tile_pool(name="psum_mm", bufs=2, space="PSUM") as pmm,
        ):
            # ---- constants ----
            c_negit = pp.tile([128, 1], f32, tag="c_negit")
            nc.gpsimd.memset(c_negit[:], -INV_T)
            c_eps = pp.tile([128, 1], f32, tag="c_eps")
            nc.gpsimd.memset(c_eps[:], float(EPS))

            # ---- DMAs: small den-critical tensors first, gt in quarters
            # (slice-granular deps let the first matmul start after 256KB),
            # then the single combined window-table broadcast ----
            gt = pp.tile([128, N], f16, tag="gt")
            mygt_s = pp.tile([128, NRB, 128], f16, tag="mygt_s")
            nc.sync.dma_start(gt[:, 0:1024], gt16.ap()[:, 0:1024])
            nc.sync.dma_start(mygt_s[:], mygt.ap())
            nc.sync.dma_start(gt[:, 1024:2048], gt16.ap()[:, 1024:2048])
            tabs = pp.tile([128, NK, 5, wfix], f16, tag="tabs")
            HT = 5 * wfix
            nc.sync.dma_start(
                tabs[:, 0], wtab.ap()[:, 0:HT].to_broadcast((128, HT)))
            wmy_s = pp.tile([128, NK, 5], f32, tag="wmy_s")
            nc.sync.dma_start(wmy_s[:], wmy.ap())
            myg_s = pp.tile([128, NRB, 128], f16, tag="myg_s")
            nc.sync.dma_start(myg_s[:], myg.ap())
            nc.sync.dma_start(gt[:, 2048:3072], gt16.ap()[:, 2048:3072])
            nc.sync.dma_start(
                tabs[:, 1], wtab.ap()[:, HT : 2 * HT].to_broadcast((128, HT)))
            nc.sync.dma_start(gt[:, 3072:4096], gt16.ap()[:, 3072:4096])
            wh_s = pp.tile([128, NK, WB, 129], f16, tag="wh_s")
            nc.sync.dma_start(wh_s[:], wh.ap())

            # ---- helpers to emit each phase piecewise so the ACT queue
            # interleaves sim work into the den exp stream ----
            denacc = pp.tile([128, NRB, NCH], f32, tag="denacc")
            denx = pp.tile([128, 4], f32, tag="denx")
            es_late = []

            def den_chunk(rb, h, dve_sum=False):
                """One [128,2048] den chunk. dve_sum: skip the ACT accum
                and let the (idle-by-then) DVE reduce the exp tile."""
                ps = pmm.tile([128, DEN_CHUNK], f32, tag="ps")
                for q in range(DEN_CHUNK // CT):
                    j0 = h * DEN_CHUNK + q * CT
                    nc.tensor.matmul(
                        ps[:, q * CT : (q + 1) * CT],
                        mygt_s[:, rb, :],
                        gt[:, j0 : j0 + CT],
                        start=True, stop=True)
                es = wp.tile([128, DEN_CHUNK], f16, tag="es", bufs=3)
                if dve_sum:
                    nc.scalar.activation(
                        es[:], ps[:], Act.Exp, scale=INV_T, bias=c_negit[:])
                    es_late.append((es, rb, h))
                else:
                    nc.scalar.activation(
                        es[:], ps[:], Act.Exp, scale=INV_T, bias=c_negit[:],
                        accum_out=denacc[:, rb, h : h + 1])

            # ---- epilogue inputs that only need myg: emit first on DVE ----
            ssq4 = pp.tile([128, NRB], f32, tag="ssq4")
            for rb in range(NRB):
                tr = wp.tile([128, 128], f16, tag="tr")
                nc.vector.scalar_tensor_tensor(
                    tr[:], myg_s[:, rb, :], 0.0, myg_s[:, rb, :],
                    Alu.bypass, Alu.mult,
                    accum_out=ssq4[:, rb : rb + 1])

            # first chunk split in half: ACT starts after 2 matmuls
            ps0 = pmm.tile([128, DEN_CHUNK], f32, tag="ps")
            for q in range(2):
                nc.tensor.matmul(
                    ps0[:, q * CT : (q + 1) * CT], mygt_s[:, 0, :],
                    gt[:, q * CT : (q + 1) * CT], start=True, stop=True)
            es0 = wp.tile([128, DEN_CHUNK], f16, tag="es", bufs=3)
            nc.scalar.activation(
                es0[:, 0:1024], ps0[:, 0:1024], Act.Exp, scale=INV_T,
                bias=c_negit[:], accum_out=denx[:, 0:1])
            for q in range(2, 4):
                nc.tensor.matmul(
                    ps0[:, q * CT : (q + 1) * CT], mygt_s[:, 0, :],
                    gt[:, q * CT : (q + 1) * CT], start=True, stop=True)
            nc.scalar.activation(
                es0[:, 1024:2048], ps0[:, 1024:2048], Act.Exp, scale=INV_T,
                bias=c_negit[:], accum_out=denx[:, 1:2])
            den_chunk(1, 0)

            # ---- sim windows (DVE): dist = sum over 5 pseudo-features of
            # |t_g - my_g| (key folded in as feature 4 = 32*dense_id) ----
            ab = pp.tile([128, NK, 5, wfix], f16, tag="ab")
            for k in range(NK):
                for g in range(5):
                    dg = wp.tile([128, wfix], f16, tag="dg")
                    nc.vector.tensor_scalar(
                        dg[:], tabs[:, k, g, :], wmy_s[:, k, g : g + 1], None,
                        Alu.subtract)
                    nc.vector.scalar_tensor_tensor(
                        ab[:, k, g, :], dg[:], -1.0, dg[:], Alu.mult, Alu.max)
            s01 = pp.tile([128, NK, wfix], f16, tag="s01")
            nc.vector.tensor_tensor(
                s01[:], ab[:, :, 0, :], ab[:, :, 1, :], Alu.add)
            s23 = pp.tile([128, NK, wfix], f16, tag="s23")
            nc.vector.tensor_tensor(
                s23[:], ab[:, :, 2, :], ab[:, :, 3, :], Alu.add)
            nc.vector.tensor_tensor(s01[:], s01[:], s23[:], Alu.add)
            dist = pp.tile([128, NK, wfix], f16, tag="dist")
            nc.vector.tensor_tensor(dist[:], s01[:], ab[:, :, 4, :], Alu.add)

            den_chunk(2, 0)
            den_chunk(3, 0)
            den_chunk(0, 1)

            # ---- sim = exp(-dist/2): one ACT op slotted mid-den ----
            sim = pp.tile([128, NK, wfix], f16, tag="sim")
            nc.scalar.activation(sim[:], dist[:], Act.Exp, scale=-0.5)
            # sim^T via XBAR DMA transpose (sync queue, overlaps den)
            simTs = []
            for k in range(NK):
                simT = pp.tile([128, WB, 128], f16, tag=f"simT{k}",
                               name=f"simT{k}")
                nc.sync.dma_start_transpose(simT[:], sim[:, k, :])
                simTs.append(simT)

            den_chunk(1, 1)
            den_chunk(2, 1)
            # last chunk split in half: the final exp gates the output
            psL = pmm.tile([128, DEN_CHUNK], f32, tag="ps")
            for q in range(2):
                nc.tensor.matmul(
                    psL[:, q * CT : (q + 1) * CT], mygt_s[:, 3, :],
                    gt[:, 2048 + q * CT : 2048 + (q + 1) * CT],
                    start=True, stop=True)
            esL = wp.tile([128, DEN_CHUNK], f16, tag="es", bufs=3)
            nc.scalar.activation(
                esL[:, 0:1024], psL[:, 0:1024], Act.Exp, scale=INV_T,
                bias=c_negit[:], accum_out=denx[:, 2:3])
            for q in range(2, 4):
                nc.tensor.matmul(
                    psL[:, q * CT : (q + 1) * CT], mygt_s[:, 3, :],
                    gt[:, 2048 + q * CT : 2048 + (q + 1) * CT],
                    start=True, stop=True)
            nc.scalar.activation(
                esL[:, 1024:2048], psL[:, 1024:2048], Act.Exp, scale=INV_T,
                bias=c_negit[:], accum_out=denx[:, 3:4])

            # ---- P = simT @ [H|1] (PE, after den matmuls) ----
            psb = pp.tile([128, NK, 129], f32, tag="psb")
            for k in range(NK):
                pps = pmm.tile([128, DEN_CHUNK], f32, tag="ps")
                for i in range(WB):
                    nc.tensor.matmul(
                        pps[:, 0:129],
                        simTs[k][:, i, :],
                        wh_s[:, k, i, :],
                        start=(i == 0), stop=(i == WB - 1))
                nc.vector.tensor_copy(psb[:, k, :], pps[:, 0:129])

            # ---- epilogue ----
            dexp = pp.tile([128, NRB], f32, tag="dexp")
            nc.scalar.activation(
                dexp[:], ssq4[:], Act.Exp, scale=INV_T, bias=c_negit[:])
            nc.vector.tensor_tensor(
                denacc[:, 0, 0:1], denx[:, 0:1], denx[:, 1:2], Alu.add)
            nc.vector.tensor_tensor(
                denacc[:, 3, 1:2], denx[:, 2:3], denx[:, 3:4], Alu.add)
            den4 = pp.tile([128, NRB], f32, tag="den4")
            nc.vector.tensor_tensor(
                den4[:], denacc[:, :, 0], denacc[:, :, 1], Alu.add)
            nc.vector.tensor_tensor(den4[:], den4[:], dexp[:], Alu.subtract)
            # S2 raw = myg . P ; S3 = ones column
            s24 = pp.tile([128, NRB], f32, tag="s24")
            s34 = pp.tile([128, NRB], f32, tag="s34")
            for rb in range(NRB):
                k = rb % NK
                tr2 = wp.tile([128, 128], f32, tag="tr2")
                nc.vector.scalar_tensor_tensor(
                    tr2[:], myg_s[:, rb, :], 0.0, psb[:, k, 0:128],
                    Alu.bypass, Alu.mult,
                    accum_out=s24[:, rb : rb + 1])
                nc.vector.tensor_scalar(
                    s34[:, rb : rb + 1], psb[:, k, 128:129], float(V), None,
                    Alu.mult)
            # same-view diag clip correction: S2 += 1 - ssq
            corr = pp.tile([128, NRB], f32, tag="corr")
            nc.vector.tensor_scalar(
                corr[:], ssq4[:], -1.0, 1.0, Alu.mult, Alu.add)
            nc.vector.tensor_tensor(s24[:], s24[:], corr[:], Alu.add)
            # raw terms out; the O(N) ln/divide epilogue runs on host
            outt = pp.tile([128, 3, NRB], f32, tag="outt")
            nc.vector.tensor_copy(outt[:, 0, :], s24[:])
            nc.vector.tensor_copy(outt[:, 1, :], s34[:])
            nc.vector.tensor_copy(outt[:, 2, :], den4[:])
            nc.sync.dma_start(out_ext.ap(), outt[:])

    return nc


_NC_CACHE = {}


def _get_nc(wfix):
    if wfix not in _NC_CACHE:
        _NC_CACHE[wfix] = _build(wfix)
    return _NC_CACHE[wfix]


def kernel(features, labels, cat_phenotypes, cont_phenotypes):
    from concourse.bass_utils import run_bass_kernel_spmd

    feats = np.asarray(features, dtype=np.float32)          # [B, V, D]
    lab = np.asarray(labels).astype(np.int64)               # [B]
    cat = np.asarray(cat_phenotypes).astype(np.int64)       # [B, 4]
    cont = np.asarray(cont_phenotypes, dtype=np.float32)    # [B, 4]

    # --- host layout prep (normalize, sort by packed key, windows) ---
    key = lab + 10 * (cat[:, 0] + 5 * (cat[:, 1] + 5 * (cat[:, 2] + 5 * cat[:, 3])))
    _, inv = np.unique(key, return_inverse=True)
    assert inv.max() < 2048, "dense key id must stay fp16-exact at *32"
    key = inv * 32  # dense id scaled: fp16-exact, |delta|>=32 when different
    order = np.argsort(key, kind="stable")
    keyS = key[order].astype(np.float32)
    contS = cont[order].astype(np.float16)                  # fp16-exact values
    gn = feats / np.linalg.norm(feats, axis=-1, keepdims=True)
    gnS = gn[order]                                         # [B, V, D]
    G = np.swapaxes(gnS, 0, 1).reshape(N, D).astype(np.float16)  # view-major
    gt16 = np.ascontiguousarray(G.T)                        # [D, N]
    H = (gnS[:, 0, :] + gnS[:, 1, :]).astype(np.float16)    # [B, D]

    # per 128-row k-block windows (aligned to 128)
    NBLK_G = B // 128
    lo = np.searchsorted(keyS, keyS[np.arange(0, B, 128)])
    hi = np.searchsorted(keyS, keyS[np.arange(127, B, 128)], side="right")
    lo128 = (lo // 128) * 128
    span = hi - lo128
    wfix = max(256, int(-(-span.max() // 128)) * 128)
    WB = wfix // 128

    keyP = np.concatenate([keyS, np.full(wfix, -1.0, np.float32)])
    contP = np.concatenate([contS, np.zeros((wfix, 4), np.float16)], axis=0)
    HP = np.concatenate([H, np.zeros((wfix, D), np.float16)], axis=0)
    onesP = np.concatenate(
        [np.ones(B, np.float16), np.zeros(wfix, np.float16)])

    in_maps = []
    for c in range(NCORES):
        mygt = np.empty((128, NRB, 128), np.float16)
        myg = np.empty((128, NRB, 128), np.float16)
        for rb in range(NRB):
            v, k = divmod(rb, NK)
            r0 = v * B + c * RB + k * 128
            mygt[:, rb, :] = gt16[:, r0 : r0 + 128]
            myg[:, rb, :] = G[r0 : r0 + 128, :]
        wh = np.empty((128, NK, WB, 129), np.float16)
        wtab = np.empty((NK, 5, wfix), np.float16)
        wmy = np.empty((128, NK, 5), np.float32)
        for k in range(NK):
            kb = c * NK + k
            s0 = int(lo128[kb])
            wtab[k, 0:4] = contP[s0 : s0 + wfix].T
            wtab[k, 4] = keyP[s0 : s0 + wfix].astype(np.float16)
            wh[:, k, :, 0:128] = HP[s0 : s0 + wfix].reshape(WB, 128, D).transpose(1, 0, 2)
            wh[:, k, :, 128] = onesP[s0 : s0 + wfix].reshape(WB, 128).T
            b0 = c * RB + k * 128
            wmy[:, k, 0:4] = contS[b0 : b0 + 128].astype(np.float32)
            wmy[:, k, 4] = keyS[b0 : b0 + 128]
        in_maps.append({
            "gt16": gt16,
            "mygt": mygt,
            "myg": myg,
            "wh": wh,
            "wtab": np.ascontiguousarray(wtab.reshape(1, NK * 5 * wfix)),
            "wmy": wmy,
        })

    nc = _get_nc(wfix)
    trace = bool(int(os.environ.get("KERNEL_TRACE", "0")))
    res = run_bass_kernel_spmd(nc, in_maps, list(range(NCORES)), trace=trace)
    if trace:
        kernel.last_exec_time_ns = res.exec_time_ns

    total = 0.0
    for c in range(NCORES):
        o = res.results[c]["out"].astype(np.float64)
        s2, s3, den = o[:, 0, :], o[:, 1, :], o[:, 2, :]
        r = ((s2 - s3) * INV_T - s3 * np.log(den + EPS)) / (s3 + EPS)
        total += float(r.sum())
    loss = -total / float(N)
    return np.float32(loss)


# revision 18
# speedup vs baseline: 1.0160x; 1.0160x over previous
"""Trainium2 Bass kernel for the APheSCL supervised-contrastive loss.

Data-parallel over anchor rows of the [N,N] logits matrix (N=V*B=4096),
256 batch rows (512 anchors) per core. v2 redesign vs the first working
kernel (148us):

 - fp16 matmuls (PE 1 cycle/row vs fp32's 4) for the N^2 gram pass.
 - softmax denominator: per 128-anchor row-block, 4 matmuls fill a
   [128,2048] PSUM span (4 banks) and ONE activation instruction does
   exp((adc-1)/T) with accum_out -> 8 big ACT ops total instead of 64
   small ones (ACT SBUF-access init is ~185ns per instruction).
 - the sim-weighted logit sum S2 = sum_j sim[a,j]*adc[a,j] is computed
   as g_a . P_a with P = Sim @ [H | 1], H_j = g_j^v0 + g_j^v1 - a tiny
   PE matmul - instead of 32 DVE scalar_tensor_tensor passes. The ones
   column gives S3 = sim row sums for free.
 - HOST-side prep (layout only, O(N*D)): rows are SORTED by the packed
   phenotype key (label + cats). sim[a,j] is nonzero only where keys
   match (plus ~e^-32 tails), so each sorted 128-anchor block's support
   is one contiguous j-window; the kernel computes sim on [128, W]
   windows (W ~ 256-512) instead of [128, 2048] - ~4-8x less DVE work.
   Window tables/H slices are gathered per core on host so the SPMD
   program stays static.
 - key equality folded into the L1 distance as a +64*(key!=mykey)
   pseudo-feature (exp(-32) ~ 1e-14 kills mismatches); |dx| built from
   validated DVE ops: d=TS(sub) then |d|=STT(d,-1,d,mult,max).
 - sim^T for the P-matmul via one 3D XBAR DMA-transpose per k-block
   (out[j,jb,a] = in[a, jb*128+j], verified on HW), zero engine time.
 - diagonal exp terms (exact, from the same fp16 values the PE saw) are
   subtracted from the denominator; same-view diag of S2 corrected to
   the reference's clip(adc)=1.

Host does normalization + fp16 cast + transposes + key packing/sorting
(O(N*D) layout prep); every O(N^2) term runs on device.
"""

import os

import numpy as np

TEMP = 0.07
INV_T = 1.0 / TEMP
EPS = 1e-8
B = 2048
V = 2
D = 128
N = V * B
NCORES = 8
RB = B // NCORES          # 256 batch rows per core
NK = RB // 128            # 2 anchor k-blocks of 128 per core
NRB = V * NK              # 4 anchor row-blocks per core (view-major)
CT = 512                  # matmul moving tile
DEN_CHUNK = 2048          # PSUM span per den exp instruction
NCH = N // DEN_CHUNK      # 2 chunks per row-block


def _patch_tile_drain():
    """This container's walrus rejects >1 sync-wait on one TPB_CTRL
    (Drain). Split the TileContext tail-drain's waits across single-wait
    SP nops (still before the all-engine barrier: semantics unchanged)."""
    from concourse import tile, mybir
    from concourse.vector_clock import ScopedClock

    if getattr(tile.TileContext, "_drain_split_patched", False):
        return

    def _drain_and_barrier(self, tick_clock, wait_clock):
        nc = self.nc
        drain_inst = nc.sync.drain()
        wait_clock.add_sem_waits(
            drain_inst.ins, ScopedClock({None: tick_clock.global_clock})
        )
        si = drain_inst.ins.sync_info
        if si is not None and si.on_wait and len(si.on_wait) > 1:
            waits = list(si.on_wait)
            si.on_wait = waits[:1]
            for w in waits[1:]:
                nop = nc.sync.nop(nofuse=True, hint="drain_split_wait")
                nsi = nop.ins.sync_info
                if nsi is None:
                    nop.ins.sync_info = mybir.SyncInfo(on_wait=[w], on_update=[])
                else:
                    nsi.on_wait = [w]
        nc.all_engine_barrier()
        assert self.sems is not None
        popped = nc._tile_sem_poison_stack.pop()
        assert popped is self._sem_poison
        nc.clear_and_free_semaphores(list(self.sems.allocated().values()))
        nc.all_engine_barrier()

    tile.TileContext._drain_and_barrier = _drain_and_barrier
    tile.TileContext._drain_split_patched = True


_MAXW = 1


def _split_waits(nc, maxw=_MAXW):
    """This walrus build rejects instructions carrying more than ~1 sync
    wait (and 0 on the DMA-transpose struct). Move excess waits onto
    same-engine nops inserted immediately before the offending
    instruction (same program point -> semantics unchanged). Engine nops
    don't flush the datapath pipeline, unlike the drains used before."""
    from concourse import mybir

    eng_map = {
        mybir.EngineType.PE: nc.tensor,
        mybir.EngineType.DVE: nc.vector,
        mybir.EngineType.Activation: nc.scalar,
        mybir.EngineType.Pool: nc.gpsimd,
        mybir.EngineType.SP: nc.sync,
    }
    for f in nc.m.functions:
        for bb in f.blocks:
            insts = bb.instructions
            i = 0
            while i < len(insts):
                ins = insts[i]
                si = ins.sync_info
                eng = getattr(ins, "engine", None)
                mw = 0 if type(ins).__name__ == "InstDmaTransposeAnt" else maxw
                if (si is not None and si.on_wait and len(si.on_wait) > mw
                        and eng in eng_map):
                    waits = list(si.on_wait)
                    si.on_wait = waits[-mw:] if mw else []
                    extra = waits[:-mw] if mw else waits
                    pre = []
                    step = max(maxw, 1)
                    for k in range(0, len(extra), step):
                        nop = eng_map[eng].nop(nofuse=True)
                        nop_ins = nop.ins
                        for fb in f.blocks:
                            if fb.instructions and fb.instructions[-1] is nop_ins:
                                fb.instructions.pop()
                                break
                        nop_ins.sync_info = mybir.SyncInfo(
                            on_wait=list(extra[k : k + step]), on_update=[])
                        pre.append(nop_ins)
                    for off, p in enumerate(pre):
                        insts.insert(i + off, p)
                    i += len(pre)
                i += 1


def _build(wfix):
    nc = _build_inner(wfix)
    _split_waits(nc)
    return nc


def _build_inner(wfix):
    from concourse import bass, tile, mybir

    _patch_tile_drain()
    f32 = mybir.dt.float32
    f16 = mybir.dt.float16
    Alu = mybir.AluOpType
    Act = mybir.ActivationFunctionType

    WB = wfix // 128

    nc = bass.Bass("TRN2", target_bir_lowering=False, debug=False,
                   num_devices=NCORES)

    gt16 = nc.declare_dram_parameter("gt16", [128, N], f16, isOutput=False)
    mygt = nc.declare_dram_parameter("mygt", [128, NRB, 128], f16, isOutput=False)
    myg = nc.declare_dram_parameter("myg", [128, NRB, 128], f16, isOutput=False)
    wh = nc.declare_dram_parameter("wh", [128, NK, WB, 129], f16, isOutput=False)
    # all 10 window-table rows (per k: 4 cont + keyid*32) in one fp16 param
    wtab = nc.declare_dram_parameter("wtab", [1, NK * 5 * wfix], f16, isOutput=False)
    wmy = nc.declare_dram_parameter("wmy", [128, NK, 5], f32, isOutput=False)
    out_ext = nc.declare_dram_parameter("out", [128, 3, NRB], f32, isOutput=True)

    with tile.TileContext(nc) as tc:
        with (
            tc.tile_pool(name="persist", bufs=1) as pp,
            tc.tile_pool(name="work", bufs=2) as wp,
            tc.tile_pool(name="psum_mm", bufs=2, space="PSUM") as pmm,
        ):
            # ---- constants ----
            c_negit = pp.tile([128, 1], f32, tag="c_negit")
            nc.gpsimd.memset(c_negit[:], -INV_T)
            c_eps = pp.tile([128, 1], f32, tag="c_eps")
            nc.gpsimd.memset(c_eps[:], float(EPS))

            # ---- DMAs: small den-critical tensors first, gt in quarters
            # (slice-granular deps let the first matmul start after 256KB),
            # then the single combined window-table broadcast ----
            # PE p-state warm-up: dummy matmuls on scratch while DMAs
            # stream (PE ramps 0.65->2.4GHz only under sustained load)
            warm = pp.tile([128, 512], f16, tag="warm")
            nc.gpsimd.memset(warm[:], 0.5)
            wps = pmm.tile([128, DEN_CHUNK], f32, tag="ps")
            for _ in range(8):
                nc.tensor.matmul(wps[:, 0:512], warm[:, 0:128], warm[:],
                                 start=True, stop=True)
            gt = pp.tile([128, N], f16, tag="gt")
            mygt_s = pp.tile([128, NRB, 128], f16, tag="mygt_s")
            nc.sync.dma_start(mygt_s[:], mygt.ap())
            nc.sync.dma_start(gt[:, 0:512], gt16.ap()[:, 0:512])
            nc.sync.dma_start(gt[:, 512:1024], gt16.ap()[:, 512:1024])
            nc.sync.dma_start(gt[:, 1024:2048], gt16.ap()[:, 1024:2048])
            tabs = pp.tile([128, NK, 5, wfix], f16, tag="tabs")
            HT = 5 * wfix
            nc.sync.dma_start(
                tabs[:, 0], wtab.ap()[:, 0:HT].to_broadcast((128, HT)))
            wmy_s = pp.tile([128, NK, 5], f32, tag="wmy_s")
            nc.sync.dma_start(wmy_s[:], wmy.ap())
            myg_s = pp.tile([128, NRB, 128], f16, tag="myg_s")
            nc.sync.dma_start(myg_s[:], myg.ap())
            nc.sync.dma_start(gt[:, 2048:3072], gt16.ap()[:, 2048:3072])
            nc.sync.dma_start(
                tabs[:, 1], wtab.ap()[:, HT : 2 * HT].to_broadcast((128, HT)))
            nc.sync.dma_start(gt[:, 3072:4096], gt16.ap()[:, 3072:4096])
            wh_s = pp.tile([128, NK, WB, 129], f16, tag="wh_s")
            nc.sync.dma_start(wh_s[:], wh.ap())

            # ---- helpers to emit each phase piecewise so the ACT queue
            # interleaves sim work into the den exp stream ----
            denacc = pp.tile([128, NRB, NCH], f32, tag="denacc")
            denx = pp.tile([128, 2], f32, tag="denx")
            es_late = []

            def den_chunk(rb, h, dve_sum=False):
                """One [128,2048] den chunk. dve_sum: skip the ACT accum
                and let the (idle-by-then) DVE reduce the exp tile."""
                ps = pmm.tile([128, DEN_CHUNK], f32, tag="ps")
                for q in range(DEN_CHUNK // CT):
                    j0 = h * DEN_CHUNK + q * CT
                    nc.tensor.matmul(
                        ps[:, q * CT : (q + 1) * CT],
                        mygt_s[:, rb, :],
                        gt[:, j0 : j0 + CT],
                        start=True, stop=True)
                es = wp.tile([128, DEN_CHUNK], f16, tag="es", bufs=3)
                if dve_sum:
                    nc.scalar.activation(
                        es[:], ps[:], Act.Exp, scale=INV_T, bias=c_negit[:])
                    es_late.append((es, rb, h))
                else:
                    nc.scalar.activation(
                        es[:], ps[:], Act.Exp, scale=INV_T, bias=c_negit[:],
                        accum_out=denacc[:, rb, h : h + 1])

            # ---- epilogue inputs that only need myg: emit first on DVE ----
            ssq4 = pp.tile([128, NRB], f32, tag="ssq4")
            for rb in range(NRB):
                tr = wp.tile([128, 128], f16, tag="tr")
                nc.vector.scalar_tensor_tensor(
                    tr[:], myg_s[:, rb, :], 0.0, myg_s[:, rb, :],
                    Alu.bypass, Alu.mult,
                    accum_out=ssq4[:, rb : rb + 1])

            # first chunk split in half: ACT starts after 2 matmuls
            ps0 = pmm.tile([128, DEN_CHUNK], f32, tag="ps")
            for q in range(2):
                nc.tensor.matmul(
                    ps0[:, q * CT : (q + 1) * CT], mygt_s[:, 0, :],
                    gt[:, q * CT : (q + 1) * CT], start=True, stop=True)
            es0 = wp.tile([128, DEN_CHUNK], f16, tag="es", bufs=3)
            nc.scalar.activation(
                es0[:, 0:1024], ps0[:, 0:1024], Act.Exp, scale=INV_T,
                bias=c_negit[:], accum_out=denx[:, 0:1])
            for q in range(2, 4):
                nc.tensor.matmul(
                    ps0[:, q * CT : (q + 1) * CT], mygt_s[:, 0, :],
                    gt[:, q * CT : (q + 1) * CT], start=True, stop=True)
            nc.scalar.activation(
                es0[:, 1024:2048], ps0[:, 1024:2048], Act.Exp, scale=INV_T,
                bias=c_negit[:], accum_out=denx[:, 1:2])
            den_chunk(1, 0)

            # ---- sim windows (DVE): dist = sum over 5 pseudo-features of
            # |t_g - my_g| (key folded in as feature 4 = 32*dense_id) ----
            ab = pp.tile([128, NK, 5, wfix], f16, tag="ab")
            for k in range(NK):
                for g in range(5):
                    dg = wp.tile([128, wfix], f16, tag="dg")
                    nc.vector.tensor_scalar(
                        dg[:], tabs[:, k, g, :], wmy_s[:, k, g : g + 1], None,
                        Alu.subtract)
                    nc.vector.scalar_tensor_tensor(
                        ab[:, k, g, :], dg[:], -1.0, dg[:], Alu.mult, Alu.max)
            s01 = pp.tile([128, NK, wfix], f16, tag="s01")
            nc.vector.tensor_tensor(
                s01[:], ab[:, :, 0, :], ab[:, :, 1, :], Alu.add)
            s23 = pp.tile([128, NK, wfix], f16, tag="s23")
            nc.vector.tensor_tensor(
                s23[:], ab[:, :, 2, :], ab[:, :, 3, :], Alu.add)
            nc.vector.tensor_tensor(s01[:], s01[:], s23[:], Alu.add)
            dist = pp.tile([128, NK, wfix], f16, tag="dist")
            nc.vector.tensor_tensor(dist[:], s01[:], ab[:, :, 4, :], Alu.add)

            den_chunk(2, 0)
            den_chunk(3, 0)
            den_chunk(0, 1)

            # ---- sim = exp(-dist/2): one ACT op slotted mid-den ----
            sim = pp.tile([128, NK, wfix], f16, tag="sim")
            nc.scalar.activation(sim[:], dist[:], Act.Exp, scale=-0.5)
            # sim^T via XBAR DMA transpose (sync queue, overlaps den)
            simTs = []
            for k in range(NK):
                simT = pp.tile([128, WB, 128], f16, tag=f"simT{k}",
                               name=f"simT{k}")
                nc.sync.dma_start_transpose(simT[:], sim[:, k, :])
                simTs.append(simT)

            den_chunk(1, 1)
            den_chunk(2, 1)
            den_chunk(3, 1)

            # ---- P = simT @ [H|1] (PE, after den matmuls) ----
            psb = pp.tile([128, NK, 129], f32, tag="psb")
            for k in range(NK):
                pps = pmm.tile([128, DEN_CHUNK], f32, tag="ps")
                for i in range(WB):
                    nc.tensor.matmul(
                        pps[:, 0:129],
                        simTs[k][:, i, :],
                        wh_s[:, k, i, :],
                        start=(i == 0), stop=(i == WB - 1))
                nc.vector.tensor_copy(psb[:, k, :], pps[:, 0:129])

            # ---- epilogue ----
            dexp = pp.tile([128, NRB], f32, tag="dexp")
            nc.scalar.activation(
                dexp[:], ssq4[:], Act.Exp, scale=INV_T, bias=c_negit[:])
            nc.vector.tensor_tensor(
                denacc[:, 0, 0:1], denx[:, 0:1], denx[:, 1:2], Alu.add)
            den4 = pp.tile([128, NRB], f32, tag="den4")
            nc.vector.tensor_tensor(
                den4[:], denacc[:, :, 0], denacc[:, :, 1], Alu.add)
            nc.vector.tensor_tensor(den4[:], den4[:], dexp[:], Alu.subtract)
            # S2 raw = myg . P ; S3 = ones column
            s24 = pp.tile([128, NRB], f32, tag="s24")
            s34 = pp.tile([128, NRB], f32, tag="s34")
            for rb in range(NRB):
                k = rb % NK
                tr2 = wp.tile([128, 128], f32, tag="tr2")
                nc.vector.scalar_tensor_tensor(
                    tr2[:], myg_s[:, rb, :], 0.0, psb[:, k, 0:128],
                    Alu.bypass, Alu.mult,
                    accum_out=s24[:, rb : rb + 1])
                nc.vector.tensor_scalar(
                    s34[:, rb : rb + 1], psb[:, k, 128:129], float(V), None,
                    Alu.mult)
            # same-view diag clip correction: S2 += 1 - ssq
            corr = pp.tile([128, NRB], f32, tag="corr")
            nc.vector.tensor_scalar(
                corr[:], ssq4[:], -1.0, 1.0, Alu.mult, Alu.add)
            nc.vector.tensor_tensor(s24[:], s24[:], corr[:], Alu.add)
            # raw terms out; the O(N) ln/divide epilogue runs on host
            outt = pp.tile([128, 3, NRB], f32, tag="outt")
            nc.vector.tensor_copy(outt[:, 0, :], s24[:])
            nc.vector.tensor_copy(outt[:, 1, :], s34[:])
            nc.vector.tensor_copy(outt[:, 2, :], den4[:])
            nc.sync.dma_start(out_ext.ap(), outt[:])

    return nc


_NC_CACHE = {}


def _get_nc(wfix):
    if wfix not in _NC_CACHE:
        _NC_CACHE[wfix] = _build(wfix)
    return _NC_CACHE[wfix]


def kernel(features, labels, cat_phenotypes, cont_phenotypes):
    from concourse.bass_utils import run_bass_kernel_spmd

    feats = np.asarray(features, dtype=np.float32)          # [B, V, D]
    lab = np.asarray(labels).astype(np.int64)               # [B]
    cat = np.asarray(cat_phenotypes).astype(np.int64)       # [B, 4]
    cont = np.asarray(cont_phenotypes, dtype=np.float32)    # [B, 4]

    # --- host layout prep (normalize, sort by packed key, windows) ---
    key = lab + 10 * (cat[:, 0] + 5 * (cat[:, 1] + 5 * (cat[:, 2] + 5 * cat[:, 3])))
    _, inv = np.unique(key, return_inverse=True)
    assert inv.max() < 2048, "dense key id must stay fp16-exact at *32"
    key = inv * 32  # dense id scaled: fp16-exact, |delta|>=32 when different
    order = np.argsort(key, kind="stable")
    keyS = key[order].astype(np.float32)
    contS = cont[order].astype(np.float16)                  # fp16-exact values
    gn = feats / np.linalg.norm(feats, axis=-1, keepdims=True)
    gnS = gn[order]                                         # [B, V, D]
    G = np.swapaxes(gnS, 0, 1).reshape(N, D).astype(np.float16)  # view-major
    gt16 = np.ascontiguousarray(G.T)                        # [D, N]
    H = (gnS[:, 0, :] + gnS[:, 1, :]).astype(np.float16)    # [B, D]

    # per 128-row k-block windows (aligned to 128)
    NBLK_G = B // 128
    lo = np.searchsorted(keyS, keyS[np.arange(0, B, 128)])
    hi = np.searchsorted(keyS, keyS[np.arange(127, B, 128)], side="right")
    lo128 = (lo // 128) * 128
    span = hi - lo128
    wfix = max(256, int(-(-span.max() // 128)) * 128)
    WB = wfix // 128

    keyP = np.concatenate([keyS, np.full(wfix, -1.0, np.float32)])
    contP = np.concatenate([contS, np.zeros((wfix, 4), np.float16)], axis=0)
    HP = np.concatenate([H, np.zeros((wfix, D), np.float16)], axis=0)
    onesP = np.concatenate(
        [np.ones(B, np.float16), np.zeros(wfix, np.float16)])

    in_maps = []
    for c in range(NCORES):
        mygt = np.empty((128, NRB, 128), np.float16)
        myg = np.empty((128, NRB, 128), np.float16)
        for rb in range(NRB):
            v, k = divmod(rb, NK)
            r0 = v * B + c * RB + k * 128
            mygt[:, rb, :] = gt16[:, r0 : r0 + 128]
            myg[:, rb, :] = G[r0 : r0 + 128, :]
        wh = np.empty((128, NK, WB, 129), np.float16)
        wtab = np.empty((NK, 5, wfix), np.float16)
        wmy = np.empty((128, NK, 5), np.float32)
        for k in range(NK):
            kb = c * NK + k
            s0 = int(lo128[kb])
            wtab[k, 0:4] = contP[s0 : s0 + wfix].T
            wtab[k, 4] = keyP[s0 : s0 + wfix].astype(np.float16)
            wh[:, k, :, 0:128] = HP[s0 : s0 + wfix].reshape(WB, 128, D).transpose(1, 0, 2)
            wh[:, k, :, 128] = onesP[s0 : s0 + wfix].reshape(WB, 128).T
            b0 = c * RB + k * 128
            wmy[:, k, 0:4] = contS[b0 : b0 + 128].astype(np.float32)
            wmy[:, k, 4] = keyS[b0 : b0 + 128]
        in_maps.append({
            "gt16": gt16,
            "mygt": mygt,
            "myg": myg,
            "wh": wh,
            "wtab": np.ascontiguousarray(wtab.reshape(1, NK * 5 * wfix)),
            "wmy": wmy,
        })

    nc = _get_nc(wfix)
    trace = bool(int(os.environ.get("KERNEL_TRACE", "0")))
    res = run_bass_kernel_spmd(nc, in_maps, list(range(NCORES)), trace=trace)
    if trace:
        kernel.last_exec_time_ns = res.exec_time_ns

    total = 0.0
    for c in range(NCORES):
        o = res.results[c]["out"].astype(np.float64)
        s2, s3, den = o[:, 0, :], o[:, 1, :], o[:, 2, :]
        r = ((s2 - s3) * INV_T - s3 * np.log(den + EPS)) / (s3 + EPS)
        total += float(r.sum())
    loss = -total / float(N)
    return np.float32(loss)
